# revision 19
# baseline (speedup 1.0000x reference)
"""Trainium2 Bass kernel for single-head attention (B=8, N=3136, C=147, D=64).

Sharding: data-parallel over batch across 8 NeuronCores (1 batch element/core).
Host-side shard prep: each core receives its batch element pre-transposed to
x^T [C, N] in fp16 (layout/pack prep only -- all FLOPs stay on device), plus
the tiny QKV weights pre-packed fp16 with the q/k blocks duplicated into both
PE partition halves.

Per-core algorithm (v3):
  Phase A: qkvT[j, n] = W_qkv.T @ x^T straight off the fat-DMA'd x^T tiles
     (fp16: 1 cycle/row at any moving size). The duplicated q/k weights put
     qT/kT in BOTH partition halves of a [128, N] tile, enabling PE row-group
     pairing in phase C. v natural comes from xT.T @ Wv per 128-wide subtile.
     PSUM evacuations are spread across engines: qT via ACT, kT via DVE,
     v_aug fp16 via GpSimd (from the fp32 v_nat32 residual copy on DVE).
  Phase C: per 512-wide i-chunk, per pair of 128-wide j-tiles:
       S^T[j, i] = kT.T @ qT  -- TWO K=64 fp16 matmuls run concurrently in
                                 disjoint PE row groups (base partitions 0/64)
       p = exp(S^T * scale)   -- one ACT call per pair ([128, 1024]), fp16 out
       o += v_aug.T @ p       -- K=128 PV accumulation split in row groups;
                                 row 64 gathers Z = sum_j p (softmax denom)
     epilogue: proj in transposed space (normalization commutes with the
     linear proj), one small PE transpose per 128 rows brings [pj | Z] to
     natural layout, then out = pj*(1/Z) + v + b via fused DVE ops. The
     previous chunk's epilogue stages are spread one per pair slot so they
     hide under the ACT-bound steady state.
  Emission is software-pipelined (PV trails S^T/exp by one pair) so the
  in-order PE never stalls on ACT.
fp16 (11-bit mantissa) beats fp32r/tf32 (10-bit) on accuracy and runs
1 cycle/row on the PE at any moving size. The residual path v_nat32 stays
fp32 (copied from the fp32 PSUM accumulation).
"""
import sys

for _p in ("/opt/trn_rl_repo",):
    if _p not in sys.path:
        sys.path.append(_p)

import numpy as np
from contextlib import ExitStack

import concourse.bass as bass
import concourse.bacc as bacc
import concourse.tile as tile
from concourse import mybir
from concourse.bass_utils import run_bass_kernel_spmd
from concourse.masks import make_identity

P = 128
SEQ = 3136        # N
CH = 147          # C
D = 64            # head dim
SCALE = D ** -0.5
NT = (SEQ + P - 1) // P          # 25 tiles of n/j (24 full + 1 of 64)
IC = 512                         # i-chunk width for attention
F32 = mybir.dt.float32
F32R = mybir.dt.float32r
F16 = mybir.dt.float16
EXP = mybir.ActivationFunctionType.Exp
COPY = mybir.ActivationFunctionType.Copy

_cache = {}


def _ichunks():
    out = []
    i0 = 0
    while i0 < SEQ:
        out.append((i0, min(IC, SEQ - i0)))
        i0 += IC
    return out


def build():
    nc = bacc.Bacc("TRN2", target_bir_lowering=False, debug=False, num_devices=8)
    # host passes x^T (fp16) and pre-packed fp16 weights (layout prep only)
    xt_d = nc.declare_dram_parameter("xt", [CH, SEQ], F16, isOutput=False)
    wq2_d = nc.declare_dram_parameter("wq2", [CH, P], F16, isOutput=False)
    wk2_d = nc.declare_dram_parameter("wk2", [CH, P], F16, isOutput=False)
    wv_d = nc.declare_dram_parameter("wv", [CH, D], F16, isOutput=False)
    wp_d = nc.declare_dram_parameter("w_proj", [D, D], F32, isOutput=False)
    bb_d = nc.declare_dram_parameter("bb", [P, D], F32, isOutput=False)
    out = nc.declare_dram_parameter("out", [SEQ, D], F32, isOutput=True)

    with ExitStack() as ctx:
        tc = ctx.enter_context(tile.TileContext(nc))
        singles = ctx.enter_context(tc.tile_pool(name="singles", bufs=1))

        # --- weights first on the (otherwise idle) GpSimd DMA queue so the
        # first qkv matmul isn't stuck behind the x^T issue train ---
        wq2_hi = singles.tile([P, P], F16)
        wq2_lo = singles.tile([CH - P, P], F16)
        wk2_hi = singles.tile([P, P], F16)
        wk2_lo = singles.tile([CH - P, P], F16)
        wv_hi = singles.tile([P, D], F16)
        wv_lo = singles.tile([CH - P, D], F16)
        wp = singles.tile([D, D], F32)
        bb = singles.tile([P, D], F32)
        nc.scalar.dma_start(out=wq2_hi, in_=wq2_d[0:P, :])
        nc.scalar.dma_start(out=wq2_lo, in_=wq2_d[P:CH, :])
        nc.scalar.dma_start(out=wk2_hi, in_=wk2_d[0:P, :])
        nc.scalar.dma_start(out=wk2_lo, in_=wk2_d[P:CH, :])
        nc.scalar.dma_start(out=wv_hi, in_=wv_d[0:P, :])
        nc.scalar.dma_start(out=wv_lo, in_=wv_d[P:CH, :])
        nc.scalar.dma_start(out=wp, in_=wp_d[:, :])
        nc.scalar.dma_start(out=bb, in_=bb_d[:, :])

        # --- fat DMA loads: x^T ---
        # split column-wise so early qkv chunks start before the full load
        # lands, and so no single DGE queue carries the whole tensor
        xt_hi = singles.tile([P, SEQ], F16)
        xt_lo = singles.tile([CH - P, SEQ], F16)
        _n0 = 0
        while _n0 < SEQ:
            _w = min(IC, SEQ - _n0)
            nc.sync.dma_start(out=xt_hi[:, _n0:_n0 + _w],
                              in_=xt_d[0:P, _n0:_n0 + _w])
            nc.scalar.dma_start(out=xt_lo[:, _n0:_n0 + _w],
                                in_=xt_d[P:CH, _n0:_n0 + _w])
            _n0 += IC

        wp_r = singles.tile([D, D], F32R)
        nc.vector.tensor_copy(wp_r, wp)

        ident = singles.tile([P, P], F32)
        make_identity(nc, ident)

        # --- big SBUF holdings ---
        qT2 = singles.tile([P, SEQ], F16)         # qT duplicated in both halves
        kT2 = singles.tile([P, SEQ], F16)         # kT duplicated in both halves
        v_aug = singles.tile([P, NT, D + 1], F16)  # v natural + ones col (PV lhsT)
        v_nat32 = singles.tile([P, NT, D], F32)    # v natural, fp32 (residual)
        # ones column written once; v writes fill the rest per subtile
        nc.vector.memset(v_aug[:, :, D:D + 1], 1.0)

        # ---------------- Phase A: qkvT + v natural ----------------
        # v is computed transposed (vT = Wv.T @ xT -- two fat 512-col matmuls
        # per chunk instead of 2 small weight-load-bound matmuls per 128-wide
        # subtile), then brought to natural layout off the PE: a block-
        # relocating SBUF->SBUF DMA places 32x32 blocks at their transposed
        # positions and one DVE StreamTranspose fixes the interiors.
        with ExitStack() as actx:
            a_mm = actx.enter_context(tc.tile_pool(name="a_mm", bufs=2, space="PSUM"))
            a_vt = actx.enter_context(tc.tile_pool(name="a_vt", bufs=2, space="PSUM"))
            a_vs = actx.enter_context(tc.tile_pool(name="a_vs", bufs=2))

            def emit_qkv(n0, csz):
                pq = a_mm.tile([P, IC], F32, name="pq", tag="pq")
                pk = a_mm.tile([P, IC], F32, name="pk", tag="pk")
                for (ps_t, whi, wlo) in ((pq, wq2_hi, wq2_lo),
                                         (pk, wk2_hi, wk2_lo)):
                    nc.tensor.matmul(ps_t[:, 0:csz], whi,
                                     xt_hi[:, n0:n0 + csz],
                                     start=True, stop=False)
                    nc.tensor.matmul(ps_t[:, 0:csz], wlo[0:CH - P, :],
                                     xt_lo[0:CH - P, n0:n0 + csz],
                                     start=False, stop=True)
                # evacuate PSUM: qT via ACT, kT via DVE (idle engines)
                nc.scalar.activation(qT2[:, n0:n0 + csz], pq[:, 0:csz], COPY)
                nc.vector.tensor_copy(kT2[:, n0:n0 + csz], pk[:, 0:csz])
                # vT = Wv.T @ xT [64, csz] -- two fat 512-col matmuls instead
                # of 2 small weight-load-bound matmuls per 128-wide subtile
                vt = a_vt.tile([D, IC], F32, name="vt", tag="vt")
                nc.tensor.matmul(vt[:, 0:csz], wv_hi, xt_hi[:, n0:n0 + csz],
                                 start=True, stop=False)
                nc.tensor.matmul(vt[:, 0:csz], wv_lo[0:CH - P, :],
                                 xt_lo[0:CH - P, n0:n0 + csz],
                                 start=False, stop=True)
                vt_sb = a_vs.tile([D, IC], F32, name="vt_sb", tag="vs")
                nc.scalar.activation(vt_sb[:, 0:csz], vt[:, 0:csz], COPY)
                # block-relocate vT -> vre so a 32x32 interior transpose
                # yields v natural: vre[32g+a, s, 32h+b] = vT[32h+a, n0+128s+32g+b]
                ns = (csz + P - 1) // P          # 128-wide subtiles
                gc = min(4, csz // 32)           # 32-col blocks per subtile
                vre = a_vs.tile([P, 4, D], F32, name="vre", tag="vr")
                src = vt_sb[:, 0:csz].rearrange("p (s g b) -> p s g b",
                                                g=gc, b=32)
                for h in range(2):
                    for g in range(gc):
                        nc.sync.dma_start(
                            out=vre[32 * g:32 * g + 32, 0:ns,
                                    32 * h:32 * h + 32],
                            in_=src[32 * h:32 * h + 32, :, g, :])
                vraw = a_vs.tile([P, 4, D], F32, name="vraw", tag="vw")
                pp = 32 * gc
                nc.vector.transpose(vraw[0:pp, 0:ns, :], vre[0:pp, 0:ns, :])
                jt0 = n0 // P
                for s in range(ns):
                    sw = min(P, csz - s * P)
                    # residual copy with the proj bias folded in (v_nat32 is
                    # only read by the epilogue residual)
                    nc.vector.tensor_add(v_nat32[0:sw, jt0 + s, :],
                                         vraw[0:sw, s, :], bb[0:sw, :])
                    nc.gpsimd.tensor_copy(v_aug[0:sw, jt0 + s, 0:D],
                                          vraw[0:sw, s, :])

            _n0 = 0
            while _n0 < SEQ:
                emit_qkv(_n0, min(IC, SEQ - _n0))
                _n0 += IC

        # ---------------- Phase C: attention ----------------
        with ExitStack() as cctx:
            st_ps = cctx.enter_context(tc.tile_pool(name="st_ps", bufs=2, space="PSUM"))
            o_ps_pool = cctx.enter_context(tc.tile_pool(name="o_ps", bufs=1, space="PSUM"))
            eps_pool = cctx.enter_context(tc.tile_pool(name="eps", bufs=2, space="PSUM"))
            p_pool = cctx.enter_context(tc.tile_pool(name="p_sb", bufs=4))
            e_sb = cctx.enter_context(tc.tile_pool(name="e_sb", bufs=2))
            o_sb = cctx.enter_context(tc.tile_pool(name="o_sb", bufs=4))
            npairs = (NT + 1) // 2    # 13: 12 full pairs + 1 single

            def emit_pv(pv):
                o_pair, p, pt, icsz = pv
                # K=128 PV split into K=64 halves in alternating PE row groups:
                # consecutive matmuls run concurrently and their weight loads
                # hide under the other half's streaming.
                o_a, o_b = o_pair
                jtA, jtB = 2 * pt, 2 * pt + 1
                if jtB < NT:
                    nc.tensor.matmul(o_a, v_aug[0:D, jtA, :], p[0:D, 0, 0:icsz],
                                     start=(jtA == 0), stop=False)
                    nc.tensor.matmul(o_b, v_aug[D:P, jtA, :], p[D:P, 0, 0:icsz],
                                     start=(jtA == 0), stop=False)
                    nc.tensor.matmul(o_a, v_aug[0:D, jtB, :], p[0:D, 1, 0:icsz],
                                     start=False, stop=False)
                    nc.tensor.matmul(o_b, v_aug[D:P, jtB, :], p[D:P, 1, 0:icsz],
                                     start=False, stop=(jtB == NT - 2))
                else:
                    jsz = SEQ - jtA * P   # 64
                    nc.tensor.matmul(o_a, v_aug[0:jsz, jtA, :],
                                     p[0:jsz, 0, 0:icsz],
                                     start=False, stop=True)

            def epilogue_stages(o_pair, i0, icsz):
                """Yield the epilogue as small closures, emitted one per pair
                slot of the NEXT i-chunk so the PE burst never starves ACT."""
                o_a, o_b = o_pair
                state = {}

                def s0():
                    stU = e_sb.tile([D + 1, IC], F32R, name="stU")[:, 0:icsz]
                    nc.vector.tensor_copy(stU, o_a)
                    nc.vector.tensor_add(stU, stU, o_b)
                    pj = eps_pool.tile([D, IC], F32, name="pj", tag="eo")[:, 0:icsz]
                    nc.tensor.matmul(pj, wp_r, stU[0:D, :], start=True, stop=True)
                    pjs = e_sb.tile([D + 1, IC], F32, name="pjs")[:, 0:icsz]
                    nc.vector.tensor_copy(pjs[0:D, :], pj)
                    nc.vector.tensor_copy(pjs[D:D + 1, :],
                                          stU[D:D + 1, :].bitcast(F32))
                    state["pjs"] = pjs

                def mk_sub(t):
                    def sub():
                        pjs = state["pjs"]
                        ncols = min(P, icsz - t * P)
                        nt_idx = (i0 + t * P) // P
                        ot = eps_pool.tile([P, D + 1], F32, name="ot", tag="eo")
                        nc.tensor.transpose(
                            ot[0:ncols, 0:D + 1], pjs[:, t * P:t * P + ncols],
                            ident[0:D + 1, 0:D + 1])
                        rz = o_sb.tile([P, 1], F32, name="rz")
                        nc.vector.reciprocal(rz[0:ncols, :],
                                             ot[0:ncols, D:D + 1])
                        res = o_sb.tile([P, D], F32, name="res")
                        nc.vector.scalar_tensor_tensor(
                            res[0:ncols, :],
                            ot[0:ncols, 0:D],
                            rz[0:ncols, :],
                            v_nat32[0:ncols, nt_idx, :],
                            op0=mybir.AluOpType.mult,
                            op1=mybir.AluOpType.add)
                        nc.sync.dma_start(
                            out=out[i0 + t * P:i0 + t * P + ncols, :],
                            in_=res[0:ncols, :])
                    return sub

                return [s0] + [mk_sub(t) for t in range((icsz + P - 1) // P)]

            # Software-pipelined: PV trails S^T/exp by one pair ACROSS chunk
            # boundaries (the next chunk's first S^T is emitted before the
            # previous chunk's last PVs, so ACT never waits on a PE backlog at
            # the boundary); the epilogue trails by one chunk with its stages
            # spread every other pair slot.
            pending_pv = None        # (o_pair, p, pt, icsz)
            pending_epi = None       # epilogue stages of previous i-chunk
            for (i0, icsz) in _ichunks():
                o_pair = (
                    o_ps_pool.tile([D + 1, IC], F32, tag="oa", name="o_a")[:, 0:icsz],
                    o_ps_pool.tile([D + 1, IC], F32, tag="ob", name="o_b")[:, 0:icsz],
                )
                for pt in range(npairs):
                    jtA, jtB = 2 * pt, 2 * pt + 1
                    pair = jtB < NT
                    st = st_ps.tile([P, 2, IC], F32, name="st")
                    p = p_pool.tile([P, 2, IC], F16, name="p")
                    jwA = min(P, SEQ - jtA * P)
                    nc.tensor.matmul(
                        st[0:jwA, 0, 0:icsz],
                        kT2[0:D, jtA * P:jtA * P + jwA],
                        qT2[0:D, i0:i0 + icsz],
                        start=True, stop=True)
                    if pair:
                        nc.tensor.matmul(
                            st[:, 1, 0:icsz],
                            kT2[D:P, jtB * P:(jtB + 1) * P],
                            qT2[D:P, i0:i0 + icsz],
                            start=True, stop=True)
                        nc.scalar.activation(p[:, :, 0:icsz], st[:, :, 0:icsz],
                                             EXP, scale=SCALE)
                    else:
                        jsz = SEQ - jtA * P
                        nc.scalar.activation(p[0:jsz, 0, 0:icsz],
                                             st[0:jsz, 0, 0:icsz],
                                             EXP, scale=SCALE)
                    if pending_pv is not None:
                        emit_pv(pending_pv)
                    pending_pv = (o_pair, p, pt, icsz)
                    if pending_epi is not None and pt % 2 == 0 \
                            and pt // 2 < len(pending_epi):
                        pending_epi[pt // 2]()
                        if pt // 2 == len(pending_epi) - 1:
                            pending_epi = None
                pending_epi = epilogue_stages(o_pair, i0, icsz)
            emit_pv(pending_pv)
            for stage in pending_epi:
                stage()

    nc.compile()
    return nc


def make_in_maps(x, W_qkv, W_proj, b_proj):
    """Host-side shard prep (layout/pack only): per-core x^T in fp16,
    duplicated q/k weight blocks, broadcast bias."""
    B = x.shape[0]
    wq2 = np.concatenate([W_qkv[:, 0:D], W_qkv[:, 0:D]], axis=1)
    wk2 = np.concatenate([W_qkv[:, D:2 * D], W_qkv[:, D:2 * D]], axis=1)
    wq2 = np.ascontiguousarray(wq2, dtype=np.float16)
    wk2 = np.ascontiguousarray(wk2, dtype=np.float16)
    wv = np.ascontiguousarray(W_qkv[:, 2 * D:3 * D], dtype=np.float16)
    wp = np.ascontiguousarray(W_proj, dtype=np.float32)
    bbv = np.ascontiguousarray(
        np.broadcast_to(np.asarray(b_proj)[None, :], (P, D)), dtype=np.float32)
    return [
        {
            "xt": np.ascontiguousarray(np.asarray(x[b]).T, dtype=np.float16),
            "wq2": wq2,
            "wk2": wk2,
            "wv": wv,
            "w_proj": wp,
            "bb": bbv,
        }
        for b in range(B)
    ]


def kernel(x, W_qkv, W_proj, b_proj):
    B = x.shape[0]
    if "nc" not in _cache:
        _cache["nc"] = build()
    nc = _cache["nc"]
    in_maps = make_in_maps(x, W_qkv, W_proj, b_proj)
    res = run_bass_kernel_spmd(nc, in_maps, core_ids=list(range(B)))
    return np.stack([res.results[b]["out"] for b in range(B)], axis=0)


if __name__ == "__main__":
    rng = np.random.default_rng(0)
    x = rng.standard_normal((8, SEQ, CH), dtype=np.float32)
    W_qkv = (rng.standard_normal((CH, 3 * D), dtype=np.float32) * CH ** -0.5)
    W_proj = (rng.standard_normal((D, D), dtype=np.float32) * D ** -0.5)
    b_proj = np.zeros(D, dtype=np.float32)
    out = kernel(x, W_qkv, W_proj, b_proj)
    print("out", out.shape, out.dtype)


# revision 22
# speedup vs baseline: 1.3247x; 1.3247x over previous
"""Trainium2 Bass kernel for single-head attention (B=8, N=3136, C=147, D=64).

Sharding: data-parallel over batch across 8 NeuronCores (1 batch element/core).
Host-side shard prep: each core receives its batch element pre-transposed to
x^T [C, N] in fp16 (layout/pack prep only -- all FLOPs stay on device), plus
the tiny QKV weights pre-packed fp16 with the q/k blocks duplicated into both
PE partition halves.

Per-core algorithm (v3):
  Phase A: qkvT[j, n] = W_qkv.T @ x^T straight off the fat-DMA'd x^T tiles
     (fp16: 1 cycle/row at any moving size). The duplicated q/k weights put
     qT/kT in BOTH partition halves of a [128, N] tile, enabling PE row-group
     pairing in phase C. v natural comes from xT.T @ Wv per 128-wide subtile.
     PSUM evacuations are spread across engines: qT via ACT, kT via DVE,
     v_aug fp16 via GpSimd (from the fp32 v_nat32 residual copy on DVE).
  Phase C: per 512-wide i-chunk, per pair of 128-wide j-tiles:
       S^T[j, i] = kT.T @ qT  -- TWO K=64 fp16 matmuls run concurrently in
                                 disjoint PE row groups (base partitions 0/64)
       p = exp(S^T * scale)   -- one ACT call per pair ([128, 1024]), fp16 out
       o += v_aug.T @ p       -- K=128 PV accumulation split in row groups;
                                 row 64 gathers Z = sum_j p (softmax denom)
     epilogue: proj in transposed space (normalization commutes with the
     linear proj), one small PE transpose per 128 rows brings [pj | Z] to
     natural layout, then out = pj*(1/Z) + v + b via fused DVE ops. The
     previous chunk's epilogue stages are spread one per pair slot so they
     hide under the ACT-bound steady state.
  Emission is software-pipelined (PV trails S^T/exp by one pair) so the
  in-order PE never stalls on ACT.
fp16 (11-bit mantissa) beats fp32r/tf32 (10-bit) on accuracy and runs
1 cycle/row on the PE at any moving size. The residual path v_nat32 stays
fp32 (copied from the fp32 PSUM accumulation).
"""
import sys

for _p in ("/opt/trn_rl_repo",):
    if _p not in sys.path:
        sys.path.append(_p)

import numpy as np
from contextlib import ExitStack

import concourse.bass as bass
import concourse.bacc as bacc
import concourse.tile as tile
from concourse import mybir
from concourse.bass_utils import run_bass_kernel_spmd
from concourse.masks import make_identity

P = 128
SEQ = 3136        # N
CH = 147          # C
D = 64            # head dim
SCALE = D ** -0.5
NT = (SEQ + P - 1) // P          # 25 tiles of n/j (24 full + 1 of 64)
IC = 512                         # i-chunk width for attention
F32 = mybir.dt.float32
F32R = mybir.dt.float32r
F16 = mybir.dt.float16
EXP = mybir.ActivationFunctionType.Exp
COPY = mybir.ActivationFunctionType.Copy

_cache = {}


def _ichunks():
    out = []
    i0 = 0
    while i0 < SEQ:
        out.append((i0, min(IC, SEQ - i0)))
        i0 += IC
    # the small remainder chunk goes FIRST: the last chunk's epilogue is the
    # serial drain at kernel end, so make that one a full 512-wide chunk whose
    # preceding-epilogue work hides under 13 full-width exps, and let the tiny
    # chunk prime the pipeline instead
    out.sort(key=lambda c: c[1])
    return out


def build():
    nc = bacc.Bacc("TRN2", target_bir_lowering=False, debug=False, num_devices=8)
    # host passes x^T (fp16) and pre-packed fp16 weights (layout prep only)
    xt_d = nc.declare_dram_parameter("xt", [CH, SEQ], F16, isOutput=False)
    wq2_d = nc.declare_dram_parameter("wq2", [CH, P], F16, isOutput=False)
    wk2_d = nc.declare_dram_parameter("wk2", [CH, P], F16, isOutput=False)
    wv_d = nc.declare_dram_parameter("wv", [CH, D], F16, isOutput=False)
    wp_d = nc.declare_dram_parameter("w_proj", [D, D], F32, isOutput=False)
    bb_d = nc.declare_dram_parameter("bb", [P, D], F32, isOutput=False)
    out = nc.declare_dram_parameter("out", [SEQ, D], F32, isOutput=True)

    with ExitStack() as ctx:
        tc = ctx.enter_context(tile.TileContext(nc))
        singles = ctx.enter_context(tc.tile_pool(name="singles", bufs=1))

        # --- weights first on the (otherwise idle) GpSimd DMA queue so the
        # first qkv matmul isn't stuck behind the x^T issue train ---
        wq2_hi = singles.tile([P, P], F16)
        wq2_lo = singles.tile([CH - P, P], F16)
        wk2_hi = singles.tile([P, P], F16)
        wk2_lo = singles.tile([CH - P, P], F16)
        wv_hi = singles.tile([P, D], F16)
        wv_lo = singles.tile([CH - P, D], F16)
        wp = singles.tile([D, D], F32)
        bb = singles.tile([P, D], F32)
        nc.scalar.dma_start(out=wq2_hi, in_=wq2_d[0:P, :])
        nc.scalar.dma_start(out=wq2_lo, in_=wq2_d[P:CH, :])
        nc.scalar.dma_start(out=wk2_hi, in_=wk2_d[0:P, :])
        nc.scalar.dma_start(out=wk2_lo, in_=wk2_d[P:CH, :])
        nc.scalar.dma_start(out=wv_hi, in_=wv_d[0:P, :])
        nc.scalar.dma_start(out=wv_lo, in_=wv_d[P:CH, :])
        nc.scalar.dma_start(out=wp, in_=wp_d[:, :])
        nc.scalar.dma_start(out=bb, in_=bb_d[:, :])

        # --- fat DMA loads: x^T ---
        # split column-wise so early qkv chunks start before the full load
        # lands, and so no single DGE queue carries the whole tensor
        xt_hi = singles.tile([P, SEQ], F16)
        xt_lo = singles.tile([CH - P, SEQ], F16)
        _n0 = 0
        while _n0 < SEQ:
            _w = min(IC, SEQ - _n0)
            nc.sync.dma_start(out=xt_hi[:, _n0:_n0 + _w],
                              in_=xt_d[0:P, _n0:_n0 + _w])
            nc.scalar.dma_start(out=xt_lo[:, _n0:_n0 + _w],
                                in_=xt_d[P:CH, _n0:_n0 + _w])
            _n0 += IC

        wp_r = singles.tile([D, D], F32R)
        nc.vector.tensor_copy(wp_r, wp)

        ident = singles.tile([P, P], F32)
        make_identity(nc, ident)

        # --- big SBUF holdings ---
        qT2 = singles.tile([P, SEQ], F16)         # qT duplicated in both halves
        kT2 = singles.tile([P, SEQ], F16)         # kT duplicated in both halves
        v_aug = singles.tile([P, NT, D + 1], F16)  # v natural + ones col (PV lhsT)
        v_nat32 = singles.tile([P, NT, D], F32)    # v natural, fp32 (residual)
        # ones column written once; v writes fill the rest per subtile
        nc.vector.memset(v_aug[:, :, D:D + 1], 1.0)

        # ---------------- Phase A: qkvT + v natural ----------------
        # v is computed transposed (vT = Wv.T @ xT -- two fat 512-col matmuls
        # per chunk instead of 2 small weight-load-bound matmuls per 128-wide
        # subtile), then brought to natural layout off the PE: a block-
        # relocating SBUF->SBUF DMA places 32x32 blocks at their transposed
        # positions and one DVE StreamTranspose fixes the interiors.
        with ExitStack() as actx:
            a_mm = actx.enter_context(tc.tile_pool(name="a_mm", bufs=2, space="PSUM"))
            a_vt = actx.enter_context(tc.tile_pool(name="a_vt", bufs=2, space="PSUM"))
            a_vs = actx.enter_context(tc.tile_pool(name="a_vs", bufs=2))

            def emit_qkv(n0, csz):
                pq = a_mm.tile([P, IC], F32, name="pq", tag="pq")
                pk = a_mm.tile([P, IC], F32, name="pk", tag="pk")
                for (ps_t, whi, wlo) in ((pq, wq2_hi, wq2_lo),
                                         (pk, wk2_hi, wk2_lo)):
                    nc.tensor.matmul(ps_t[:, 0:csz], whi,
                                     xt_hi[:, n0:n0 + csz],
                                     start=True, stop=False)
                    nc.tensor.matmul(ps_t[:, 0:csz], wlo[0:CH - P, :],
                                     xt_lo[0:CH - P, n0:n0 + csz],
                                     start=False, stop=True)
                # evacuate PSUM on DVE -- ACT is the kernel-wide bottleneck
                # (exp), so keep it exp-only
                nc.vector.tensor_copy(qT2[:, n0:n0 + csz], pq[:, 0:csz])
                nc.vector.tensor_copy(kT2[:, n0:n0 + csz], pk[:, 0:csz])
                # v natural per 128-wide n-subtile: vn = xT.T @ Wv (fp16)
                nsub = (csz + P - 1) // P
                for s in range(nsub):
                    sb = n0 + s * P
                    sw = min(P, n0 + csz - sb)
                    jt = sb // P
                    vn = a_vt.tile([P, D], F32, name="vn", tag="vn")
                    nc.tensor.matmul(vn[0:sw, :],
                                     xt_hi[:, sb:sb + sw],
                                     wv_hi, start=True, stop=False)
                    nc.tensor.matmul(vn[0:sw, :],
                                     xt_lo[0:CH - P, sb:sb + sw],
                                     wv_lo[0:CH - P, :],
                                     start=False, stop=True)
                    # residual copy with the proj bias folded in (v_nat32 is
                    # only read by the epilogue residual)
                    nc.vector.tensor_add(v_nat32[0:sw, jt, :], vn[0:sw, :],
                                         bb[0:sw, :])
                    nc.vector.tensor_copy(v_aug[0:sw, jt, 0:D], vn[0:sw, :])

            _n0 = 0
            while _n0 < SEQ:
                emit_qkv(_n0, min(IC, SEQ - _n0))
                _n0 += IC

        # ---------------- Phase C: attention ----------------
        with ExitStack() as cctx:
            st_ps = cctx.enter_context(tc.tile_pool(name="st_ps", bufs=2, space="PSUM"))
            o_ps_pool = cctx.enter_context(tc.tile_pool(name="o_ps", bufs=1, space="PSUM"))
            eps_pool = cctx.enter_context(tc.tile_pool(name="eps", bufs=2, space="PSUM"))
            p_pool = cctx.enter_context(tc.tile_pool(name="p_sb", bufs=4))
            e_sb = cctx.enter_context(tc.tile_pool(name="e_sb", bufs=2))
            o_sb = cctx.enter_context(tc.tile_pool(name="o_sb", bufs=4))
            npairs = (NT + 1) // 2    # 13: 12 full pairs + 1 single

            def emit_pv(pv):
                o_pair, p, pt, icsz = pv
                # K=128 PV split into K=64 halves in alternating PE row groups:
                # consecutive matmuls run concurrently and their weight loads
                # hide under the other half's streaming.
                o_a, o_b = o_pair
                jtA, jtB = 2 * pt, 2 * pt + 1
                if jtB < NT:
                    nc.tensor.matmul(o_a, v_aug[0:D, jtA, :], p[0:D, 0, 0:icsz],
                                     start=(jtA == 0), stop=False)
                    nc.tensor.matmul(o_b, v_aug[D:P, jtA, :], p[D:P, 0, 0:icsz],
                                     start=(jtA == 0), stop=False)
                    nc.tensor.matmul(o_a, v_aug[0:D, jtB, :], p[0:D, 1, 0:icsz],
                                     start=False, stop=False)
                    nc.tensor.matmul(o_b, v_aug[D:P, jtB, :], p[D:P, 1, 0:icsz],
                                     start=False, stop=(jtB == NT - 2))
                else:
                    jsz = SEQ - jtA * P   # 64
                    nc.tensor.matmul(o_a, v_aug[0:jsz, jtA, :],
                                     p[0:jsz, 0, 0:icsz],
                                     start=False, stop=True)

            def epilogue_stages(o_pair, i0, icsz):
                """Yield the epilogue as small closures, emitted one per pair
                slot of the NEXT i-chunk so the PE burst never starves ACT."""
                o_a, o_b = o_pair
                state = {}

                def s0():
                    stU = e_sb.tile([D + 1, IC], F32R, name="stU")[:, 0:icsz]
                    nc.vector.tensor_copy(stU, o_a)
                    nc.vector.tensor_add(stU, stU, o_b)
                    pj = eps_pool.tile([D, IC], F32, name="pj", tag="eo")[:, 0:icsz]
                    nc.tensor.matmul(pj, wp_r, stU[0:D, :], start=True, stop=True)
                    pjs = e_sb.tile([D + 1, IC], F32, name="pjs")[:, 0:icsz]
                    nc.vector.tensor_copy(pjs[0:D, :], pj)
                    nc.vector.tensor_copy(pjs[D:D + 1, :],
                                          stU[D:D + 1, :].bitcast(F32))
                    state["pjs"] = pjs

                def mk_sub(t):
                    def sub():
                        pjs = state["pjs"]
                        ncols = min(P, icsz - t * P)
                        nt_idx = (i0 + t * P) // P
                        ot = eps_pool.tile([P, D + 1], F32, name="ot", tag="eo")
                        nc.tensor.transpose(
                            ot[0:ncols, 0:D + 1], pjs[:, t * P:t * P + ncols],
                            ident[0:D + 1, 0:D + 1])
                        rz = o_sb.tile([P, 1], F32, name="rz")
                        nc.vector.reciprocal(rz[0:ncols, :],
                                             ot[0:ncols, D:D + 1])
                        res = o_sb.tile([P, D], F32, name="res")
                        nc.vector.scalar_tensor_tensor(
                            res[0:ncols, :],
                            ot[0:ncols, 0:D],
                            rz[0:ncols, :],
                            v_nat32[0:ncols, nt_idx, :],
                            op0=mybir.AluOpType.mult,
                            op1=mybir.AluOpType.add)
                        nc.sync.dma_start(
                            out=out[i0 + t * P:i0 + t * P + ncols, :],
                            in_=res[0:ncols, :])
                    return sub

                return [s0] + [mk_sub(t) for t in range((icsz + P - 1) // P)]

            # Software-pipelined: PV trails S^T/exp by one pair ACROSS chunk
            # boundaries (the next chunk's first S^T is emitted before the
            # previous chunk's last PVs, so ACT never waits on a PE backlog at
            # the boundary); the epilogue trails by one chunk with its stages
            # spread every other pair slot.
            pending_pv = None        # (o_pair, p, pt, icsz)
            pending_epi = None       # epilogue stages of previous i-chunk
            for (i0, icsz) in _ichunks():
                o_pair = (
                    o_ps_pool.tile([D + 1, IC], F32, tag="oa", name="o_a")[:, 0:icsz],
                    o_ps_pool.tile([D + 1, IC], F32, tag="ob", name="o_b")[:, 0:icsz],
                )
                for pt in range(npairs):
                    jtA, jtB = 2 * pt, 2 * pt + 1
                    pair = jtB < NT
                    st = st_ps.tile([P, 2, IC], F32, name="st")
                    p = p_pool.tile([P, 2, IC], F16, name="p")
                    jwA = min(P, SEQ - jtA * P)
                    nc.tensor.matmul(
                        st[0:jwA, 0, 0:icsz],
                        kT2[0:D, jtA * P:jtA * P + jwA],
                        qT2[0:D, i0:i0 + icsz],
                        start=True, stop=True)
                    if pair:
                        nc.tensor.matmul(
                            st[:, 1, 0:icsz],
                            kT2[D:P, jtB * P:(jtB + 1) * P],
                            qT2[D:P, i0:i0 + icsz],
                            start=True, stop=True)
                        nc.scalar.activation(p[:, :, 0:icsz], st[:, :, 0:icsz],
                                             EXP, scale=SCALE)
                    else:
                        jsz = SEQ - jtA * P
                        nc.scalar.activation(p[0:jsz, 0, 0:icsz],
                                             st[0:jsz, 0, 0:icsz],
                                             EXP, scale=SCALE)
                    if pending_pv is not None:
                        emit_pv(pending_pv)
                    pending_pv = (o_pair, p, pt, icsz)
                    if pending_epi is not None and pt % 2 == 0 \
                            and pt // 2 < len(pending_epi):
                        pending_epi[pt // 2]()
                        if pt // 2 == len(pending_epi) - 1:
                            pending_epi = None
                pending_epi = epilogue_stages(o_pair, i0, icsz)
            emit_pv(pending_pv)
            for stage in pending_epi:
                stage()

    nc.compile()
    return nc


def make_in_maps(x, W_qkv, W_proj, b_proj):
    """Host-side shard prep (layout/pack only): per-core x^T in fp16,
    duplicated q/k weight blocks, broadcast bias."""
    B = x.shape[0]
    wq2 = np.concatenate([W_qkv[:, 0:D], W_qkv[:, 0:D]], axis=1)
    wk2 = np.concatenate([W_qkv[:, D:2 * D], W_qkv[:, D:2 * D]], axis=1)
    wq2 = np.ascontiguousarray(wq2, dtype=np.float16)
    wk2 = np.ascontiguousarray(wk2, dtype=np.float16)
    wv = np.ascontiguousarray(W_qkv[:, 2 * D:3 * D], dtype=np.float16)
    wp = np.ascontiguousarray(W_proj, dtype=np.float32)
    bbv = np.ascontiguousarray(
        np.broadcast_to(np.asarray(b_proj)[None, :], (P, D)), dtype=np.float32)
    return [
        {
            "xt": np.ascontiguousarray(np.asarray(x[b]).T, dtype=np.float16),
            "wq2": wq2,
            "wk2": wk2,
            "wv": wv,
            "w_proj": wp,
            "bb": bbv,
        }
        for b in range(B)
    ]


def kernel(x, W_qkv, W_proj, b_proj):
    B = x.shape[0]
    if "nc" not in _cache:
        _cache["nc"] = build()
    nc = _cache["nc"]
    in_maps = make_in_maps(x, W_qkv, W_proj, b_proj)
    res = run_bass_kernel_spmd(nc, in_maps, core_ids=list(range(B)))
    return np.stack([res.results[b]["out"] for b in range(B)], axis=0)


if __name__ == "__main__":
    rng = np.random.default_rng(0)
    x = rng.standard_normal((8, SEQ, CH), dtype=np.float32)
    W_qkv = (rng.standard_normal((CH, 3 * D), dtype=np.float32) * CH ** -0.5)
    W_proj = (rng.standard_normal((D, D), dtype=np.float32) * D ** -0.5)
    b_proj = np.zeros(D, dtype=np.float32)
    out = kernel(x, W_qkv, W_proj, b_proj)
    print("out", out.shape, out.dtype)
